# revision 2
# baseline (speedup 1.0000x reference)
"""DCNv2 (deformable conv v2) Trainium2 kernel — 8-core SPMD, batch x
H-half sharding.

v3 over v2 baseline: float8_e3m4 corner-block gather (halves gather DMA,
36MB -> 18MB/core) with mixed-dtype fp8xf16 diag-blend matmuls; idx wrap via
PE transposes + DVE permute instead of 11 serial 2B-descriptor DMAs; PE
warmup so the om conv runs at full pstate; per-tap blend groups emitted
before main GEMMs (colsT copies overlap later blends); om conv input loads
phase-split and ordered by critical path; outputs copied Act/DVE in
parallel. rel_err ~1.25e-2 (fp8 x-quantization, deterministic).
"""
import numpy as np
from contextlib import ExitStack

import concourse.bass as bass
import concourse.mybir as mybir
import concourse.tile as tile
from concourse import bacc
from concourse.bass_utils import run_bass_kernel_spmd

F16 = np.float16
GATHER_FP8 = True   # float8_e3m4 xblk + mixed-dtype blend matmuls
try:
    import ml_dtypes
    E3M4 = ml_dtypes.float8_e3m4
except ImportError:
    E3M4 = None

B, C, H, W = 4, 256, 64, 64
O = 256
K = 3
KK = 9
NCORES = 8
ROWS_PER_CORE = 32
PIX_PER_CORE = ROWS_PER_CORE * W
PH_PIX = 1024          # pixels per phase (16 rows)
NBLK_Y = 37            # block-grid rows per core
NBLK_X = 66
NBLK = NBLK_Y * NBLK_X  # 2442

f16 = mybir.dt.float16
f8e3 = mybir.dt.float8e3
f32 = mybir.dt.float32
_FP8 = GATHER_FP8 and E3M4 is not None
GDT = f8e3 if _FP8 else f16
GNP = E3M4 if _FP8 else F16
i16 = mybir.dt.int16
i32 = mybir.dt.int32
Alu = mybir.AluOpType
Act = mybir.ActivationFunctionType


def host_prep(x, weight, bias, w_om, b_om):
    """Per-core inputs. x:[B,C,H,W] f32, weight:[O,C,3,3], bias:[O],
    w_om:[27,C,3,3], b_om:[27]."""
    # weight reorder: wT_sb[p, t*256 + o] = weight[o, ch, ky, kx],
    # t = k*2 + chh, ch = chh*128 + p
    wr = weight.reshape(O, C, KK).transpose(2, 1, 0).reshape(KK * C, O)
    wT_sb = wr.reshape(18, 128, O).transpose(1, 0, 2).reshape(128, 18 * O).astype(F16)
    # om weights, chh-major: t2 = chh*9 + k (chh-major accumulation order)
    womr = w_om.reshape(27, C, KK).transpose(2, 1, 0).reshape(KK, 2, 128, 27)
    womT_sb = womr.transpose(2, 1, 0, 3).reshape(128, 18 * 27).astype(F16)
    bias_sb = bias.reshape(2, 128).T.astype(np.float32)         # [128, 2]
    bom_sb = b_om.reshape(1, 27).astype(F16)                    # [1, 27]
    ident16 = np.eye(128, dtype=F16)

    per_core = []
    for core in range(NCORES):
        b, h = divmod(core, 2)
        xb = x[b]  # [C, H, W]
        # xp: 66x66 zero-padded channel-major f16, rows [32h-1, 32h+33) of the
        # padded frame (covers the core's 32 output rows + om-conv halo)
        xp = np.zeros((C, 66, 66), dtype=F16)
        xp[:, 1:65, 1:65] = xb.astype(F16)
        xpc = xp[:, 32 * h:32 * h + 34, :]
        xpad16 = np.ascontiguousarray(xpc).reshape(2, 128, 34 * 66)
        # xblk: 2x2 corner blocks. xblk[Yb, X, a, b2, c] = xpp[c, Yb+R0+1+a, X+b2]
        # where block (Yb, X) is anchor (y0, x0) = (R0+Yb, X-1).
        R0 = max(-1, 32 * h - 4)
        xpp = np.zeros((C, 67, 67), dtype=F16)
        xpp[:, 1:65, 1:65] = xb.astype(F16)
        xblk = np.empty((NBLK_Y, NBLK_X, 2, 2, C), dtype=F16)
        for a in range(2):
            for b2 in range(2):
                xblk[:, :, a, b2, :] = xpp[:, R0 + 1 + a:R0 + 1 + a + NBLK_Y,
                                           b2:b2 + NBLK_X].transpose(1, 2, 0)
        xblk = np.ascontiguousarray(xblk).reshape(NBLK, 1024).astype(GNP)
        cconst = np.full((128, 1), -(66.0 * (64 + R0) + 63.0), dtype=np.float32)
        # grids (absolute y incl 32h): raster pixel j = 64 r + c within phase;
        # part = 64(r%2)+c, slot = r//2
        part = np.arange(128)
        kk = np.arange(KK)
        ky, kx = kk // K, kk % K
        r2 = part // 64
        ccol = part % 64
        slot = np.arange(8)
        gyk = np.zeros((128, 2, 8, KK), dtype=np.float32)
        for ph in range(2):
            gyk[:, ph, :, :] = ((32 * h + 16 * ph + r2 - 1)[:, None, None]
                                + 2 * slot[None, :, None] + ky[None, None, :])
        gxk = np.broadcast_to((ccol - 1)[:, None, None] + kx[None, None, :],
                              (128, 8, KK)).astype(np.float32).copy()
        xp0 = xpad16[0].reshape(128, 34, 66)
        xp1 = xpad16[1].reshape(128, 34, 66)
        per_core.append(dict(
            xblk=xblk,
            xp0a=np.ascontiguousarray(xp0[:, :20]),
            xp1a=np.ascontiguousarray(xp1[:, :20]),
            xp0b=np.ascontiguousarray(xp0[:, 20:]),
            xp1b=np.ascontiguousarray(xp1[:, 20:]),
            wT=wT_sb, womT=womT_sb, bias=bias_sb, bom=bom_sb,
            gyk=gyk, gxk=gxk, ident=ident16, identf32=np.eye(128, dtype=np.float32),
            cconst=cconst,
        ))
    return per_core


def build_nc(skip_compile=False):
    nc = bacc.Bacc("TRN2", target_bir_lowering=False, debug=False, num_devices=8)

    xblk_d = nc.dram_tensor("xblk", [NBLK, 1024], GDT, kind="ExternalInput")
    xpa_d = [nc.dram_tensor(f"xp{i}a", [128, 20, 66], f16, kind="ExternalInput")
             for i in range(2)]
    xpb_d = [nc.dram_tensor(f"xp{i}b", [128, 14, 66], f16, kind="ExternalInput")
             for i in range(2)]
    wT_d = nc.dram_tensor("wT", [128, 18 * 256], f16, kind="ExternalInput")
    womT_d = nc.dram_tensor("womT", [128, 18 * 27], f16, kind="ExternalInput")
    bias_d = nc.dram_tensor("bias", [128, 2], f32, kind="ExternalInput")
    bom_d = nc.dram_tensor("bom", [1, 27], f16, kind="ExternalInput")
    gyk_d = nc.dram_tensor("gyk", [128, 2, 8, KK], f32, kind="ExternalInput")
    gxk_d = nc.dram_tensor("gxk", [128, 8, KK], f32, kind="ExternalInput")
    ident_d = nc.dram_tensor("ident", [128, 128], f16, kind="ExternalInput")
    identf32_d = nc.dram_tensor("identf32", [128, 128], f32, kind="ExternalInput")
    cconst_d = nc.dram_tensor("cconst", [128, 1], f32, kind="ExternalInput")
    out_d = nc.dram_tensor("out", [256, 2048], f16, kind="ExternalOutput")

    # block view for the gather: row i -> 1024 contiguous f16 (2x2x256 block)
    xblk_ap = bass.AP(tensor=xblk_d, offset=0, ap=[[1024, NBLK], [1, 1024]])

    with tile.TileContext(nc) as tc, ExitStack() as ctx:
        const = ctx.enter_context(tc.tile_pool(name="const", bufs=1))
        xp = [const.tile([128, 34, 66], f16, name=f"xp{i}", tag=f"xp{i}") for i in range(2)]
        womT = const.tile([128, 18 * 27], f16, name="womT", tag="womT")
        gyk_t = const.tile([128, 2, 8, KK], f32, name="gyk", tag="gyk")
        gxk_t = const.tile([128, 8, KK], f32, name="gxk", tag="gxk")
        ident_t = const.tile([128, 128], f16, name="ident", tag="ident")
        identf32 = const.tile([128, 128], f32, name="identf32", tag="identf32")
        bias_t = const.tile([128, 2], f32, name="bias", tag="bias")
        cconst_t = const.tile([128, 1], f32, name="cconst", tag="cconst")
        bom_t = const.tile([1, 27], f16, name="bom", tag="bom")
        wT = const.tile([128, 18 * 256], f16, name="wT", tag="wT")
        ones_t = const.tile([1, 128], f16, name="ones", tag="ones")
        ones_px = const.tile([1, PH_PIX], f16, name="onespx", tag="onespx")
        # critical-path loads first: om ph0 needs xp0a+womT (chh-major), then
        # xp1a; bom gates the per-pt bias matmul; gyk/gxk/cconst gate idx math
        nc.sync.dma_start(out=xp[0][:, :20], in_=xpa_d[0].ap())
        nc.scalar.dma_start(out=womT[:], in_=womT_d.ap())
        nc.scalar.dma_start(out=bom_t[:], in_=bom_d.ap())
        nc.sync.dma_start(out=xp[1][:, :20], in_=xpa_d[1].ap())
        nc.scalar.dma_start(out=gyk_t[:], in_=gyk_d.ap())
        nc.scalar.dma_start(out=gxk_t[:], in_=gxk_d.ap())
        nc.scalar.dma_start(out=cconst_t[:], in_=cconst_d.ap())
        nc.scalar.dma_start(out=ident_t[:], in_=ident_d.ap())
        nc.scalar.dma_start(out=identf32[:], in_=identf32_d.ap())
        nc.sync.dma_start(out=xp[0][:, 20:], in_=xpb_d[0].ap())
        nc.sync.dma_start(out=xp[1][:, 20:], in_=xpb_d[1].ap())
        nc.scalar.dma_start(out=bias_t[:], in_=bias_d.ap())
        nc.scalar.dma_start(out=wT[:], in_=wT_d.ap())
        nc.gpsimd.memset(ones_t[:], 1.0)
        nc.gpsimd.memset(ones_px[:], 1.0)
        warm = const.tile([1, 128], f16, name="warm", tag="warm")
        nc.gpsimd.memset(warm[:], 0.0)

        # psum pools: colp 4 (om psum borrows the first two ring slots, freed
        # after the om->sbuf copies) + outp 4 = 8 banks
        colp_pool = ctx.enter_context(tc.tile_pool(name="colp", bufs=4, space="PSUM"))
        outp_pool = ctx.enter_context(tc.tile_pool(name="outp", bufs=1, space="PSUM"))

        math_pool = ctx.enter_context(tc.tile_pool(name="math", bufs=2))
        idxw_pool = ctx.enter_context(tc.tile_pool(name="idxw", bufs=2))
        g_pool = ctx.enter_context(tc.tile_pool(name="g", bufs=4))
        diag_pool = ctx.enter_context(tc.tile_pool(name="diag", bufs=48))
        cols_pool = ctx.enter_context(tc.tile_pool(name="cols", bufs=8))
        outs_pool = ctx.enter_context(tc.tile_pool(name="outs", bufs=2))

        # ---------- per-phase: om conv -> offset math -> idx -> gathers ----
        def pe_warmup():
            """~3us of dummy matmuls so om starts at full PE pstate."""
            wp = colp_pool.tile([128, 512], f32, name="warmp", tag="pc")
            for i in range(15):
                nc.tensor.matmul(wp[:, 0:128], lhsT=warm[:], rhs=warm[:],
                                 start=True, stop=True, skip_group_check=True)

        def om_conv(ph):
            """Row-paired om conv, chh-major accumulation (womT t2 = chh*9+k).
            psum_om[64*r2+c, pt, :] accumulates over (chh, k)."""
            pcb = colp_pool.tile([128, 512], f32, name=f"pom{ph}", tag="pc")
            b_ = pcb[:]
            psum_om = bass.AP(tensor=b_.tensor, offset=b_.offset,
                              ap=[b_.ap[0], [27, 8], [1, 27]])
            # contiguous accumulation per psum region (interleaved groups
            # corrupt psum on HW); chh-major t2 order so chh=0 runs first
            for pt in range(8):
                for r2 in range(2):
                    for t2 in range(18):
                        chh, k = divmod(t2, 9)
                        ky, kx = k // 3, k % 3
                        r0 = 16 * ph + 2 * pt + r2 + ky
                        lhsT = xp[chh][:, r0, kx:kx + 64]
                        nc.tensor.matmul(
                            psum_om[64 * r2:64 * r2 + 64, pt],
                            lhsT=lhsT,
                            rhs=womT[:, t2 * 27:(t2 + 1) * 27],
                            start=(t2 == 0), stop=False)
                nc.tensor.matmul(psum_om[:, pt], lhsT=ones_t[:],
                                 rhs=bom_t[:], start=False, stop=True)
            om = math_pool.tile([128, 8, 27], f32, name=f"ompm{ph}", tag=f"ompm{ph}")
            nc.scalar.activation(om[:], psum_om, Act.Copy)
            return om

        def idx_math_crit(ph, ompm):
            """Critical path: offsets -> floor -> block idx (i16, k-major).
            Only what the gather wraps need; lerp weights computed later."""
            def mt(tag):
                return math_pool.tile([128, 8, KK], f32, name=tag + str(ph),
                                      tag=tag + str(ph))

            dy = ompm[:, :, 0:KK]
            dx = ompm[:, :, KK:2 * KK]
            ty, tx_ = mt("ty"), mt("tx")
            y064, x064 = mt("y064"), mt("x064")
            ya, xa, idxf = mt("ya"), mt("xa"), mt("idxf")

            V = nc.vector
            V.tensor_tensor(ty[:], dy, gyk_t[:, ph], Alu.add)
            V.tensor_scalar_add(ty[:], ty[:], 64.0)
            V.tensor_tensor(tx_[:], dx, gxk_t[:], Alu.add)
            V.tensor_scalar_add(tx_[:], tx_[:], 64.0)
            yi32 = math_pool.tile([128, 8, KK], i32, name=f"yi32{ph}", tag=f"yi32{ph}")
            xi32 = math_pool.tile([128, 8, KK], i32, name=f"xi32{ph}", tag=f"xi32{ph}")
            yif, xif = mt("yif"), mt("xif")
            gq, gqx = mt("gq"), mt("gqx")
            V.tensor_copy(yi32[:], ty[:])
            V.tensor_copy(yif[:], yi32[:])
            V.tensor_tensor(gq[:], yif[:], ty[:], Alu.is_gt)
            V.tensor_tensor(y064[:], yif[:], gq[:], Alu.subtract)
            V.tensor_copy(xi32[:], tx_[:])
            V.tensor_copy(xif[:], xi32[:])
            V.tensor_tensor(gqx[:], xif[:], tx_[:], Alu.is_gt)
            V.tensor_tensor(x064[:], xif[:], gqx[:], Alu.subtract)
            # block idx = (ya-64-R0)*66 + (xa-63) = 66*ya + xa + cconst
            V.tensor_scalar(ya[:], y064[:], 63.0, 128.0, Alu.max, Alu.min)
            V.tensor_scalar(xa[:], x064[:], 63.0, 128.0, Alu.max, Alu.min)
            V.scalar_tensor_tensor(idxf[:], ya[:], 66.0, xa[:], Alu.mult, Alu.add)
            V.tensor_scalar(idxf[:], idxf[:], cconst_t[:, 0:1], None, Alu.add)
            return idxf, (ty, tx_, y064, x064)

        def idx_math_wc(ph, ompm, saved):
            """Deferred: bilinear corner weights x mask (needed at first
            gather completion, ~6us after the wraps)."""
            ty, tx_, y064, x064 = saved

            def mt(tag):
                return math_pool.tile([128, 8, KK], f32, name=tag + str(ph),
                                      tag=tag + str(ph))

            ml = ompm[:, :, 2 * KK:3 * KK]
            fry, frx, m_t = mt("fry"), mt("frx"), mt("m")
            s0y, g0, ay0, ay0m = mt("s0y"), mt("g0"), mt("ay0"), mt("ay0m")
            g1, ay1, ay1m = mt("g1"), mt("ay1"), mt("ay1m")
            s0x, g0x, bx0, g1x, bx1 = mt("s0x"), mt("g0x"), mt("bx0"), mt("g1x"), mt("bx1")
            wc = [mt(f"wc{i}") for i in range(4)]

            V = nc.vector
            V.tensor_tensor(fry[:], ty[:], y064[:], Alu.subtract)
            V.tensor_tensor(frx[:], tx_[:], x064[:], Alu.subtract)
            nc.scalar.activation(m_t[:], ml, Act.Sigmoid)
            V.tensor_scalar(s0y[:], fry[:], -1.0, 1.0, Alu.mult, Alu.add)
            V.scalar_tensor_tensor(g0[:], y064[:], 64.0, s0y[:], Alu.is_ge, Alu.mult)
            V.scalar_tensor_tensor(ay0[:], y064[:], 127.0, g0[:], Alu.is_le, Alu.mult)
            V.tensor_tensor(ay0m[:], ay0[:], m_t[:], Alu.mult)
            V.scalar_tensor_tensor(g1[:], y064[:], 63.0, fry[:], Alu.is_ge, Alu.mult)
            V.scalar_tensor_tensor(ay1[:], y064[:], 126.0, g1[:], Alu.is_le, Alu.mult)
            V.tensor_tensor(ay1m[:], ay1[:], m_t[:], Alu.mult)
            V.tensor_scalar(s0x[:], frx[:], -1.0, 1.0, Alu.mult, Alu.add)
            V.scalar_tensor_tensor(g0x[:], x064[:], 64.0, s0x[:], Alu.is_ge, Alu.mult)
            V.scalar_tensor_tensor(bx0[:], x064[:], 127.0, g0x[:], Alu.is_le, Alu.mult)
            V.scalar_tensor_tensor(g1x[:], x064[:], 63.0, frx[:], Alu.is_ge, Alu.mult)
            V.scalar_tensor_tensor(bx1[:], x064[:], 126.0, g1x[:], Alu.is_le, Alu.mult)
            V.tensor_tensor(wc[0][:], ay0m[:], bx0[:], Alu.mult)
            V.tensor_tensor(wc[1][:], ay0m[:], bx1[:], Alu.mult)
            V.tensor_tensor(wc[2][:], ay1m[:], bx0[:], Alu.mult)
            V.tensor_tensor(wc[3][:], ay1m[:], bx1[:], Alu.mult)
            return wc

        def idx_wrap(ph, idxf):
            """idxw[pw, k, s, pq] = round(idxf[16 pq + pw, s, k]) via PE
            transposes (PE is idle here) + 2 DVE permute-copies, then 7
            replica DMAs for the other gather-core windows."""
            idxw = idxw_pool.tile([128, KK, 8, 8], i16, name=f"idxw{ph}",
                                  tag=f"idxw{ph}")
            # T1 = idxf^T : [72 (s,k), 128 pixels]
            t1p = colp_pool.tile([72, 128], f32, name=f"t1p{ph}", tag="pc")
            i_ = idxf[:]
            src128 = bass.AP(tensor=i_.tensor, offset=i_.offset,
                             ap=[i_.ap[0], [1, 72]])
            nc.tensor.transpose(t1p[:], src128, identf32[:])
            t1s = math_pool.tile([72, 128], f32, name=f"t1s{ph}",
                                 tag=f"t1s{ph}")
            nc.scalar.activation(t1s[:], t1p[:], Act.Copy)
            # T2_pq = T1[:, 16 pq : 16 pq+16]^T : [16 pw, 72 (s,k)]
            t2 = [colp_pool.tile([16, 4, 72], f32, name=f"t2{ph}{i}", tag="pc")
                  for i in range(2)]
            for pq in range(8):
                nc.tensor.transpose(t2[pq // 4][:, pq % 4],
                                    t1s[:, 16 * pq:16 * pq + 16],
                                    identf32[0:72, 0:72])
            # assemble window 0: dst (k, s, pq) <- src (pq, s, k), cast i16
            for i in range(2):
                s_ = t2[i][:]
                srcap = bass.AP(tensor=s_.tensor, offset=s_.offset,
                                ap=[s_.ap[0], [72, 4], [9, 8], [1, 9]])
                d_ = idxw[0:16]
                dstap = bass.AP(tensor=d_.tensor, offset=d_.offset + 4 * i,
                                ap=[d_.ap[0], [1, 4], [8, 8], [64, KK]])
                nc.vector.tensor_copy(dstap, srcap)
            # each of the 8 gather cores reads its own 16-partition idx
            # window: replicate group 0 -> 1..7 (7 parallel DMAs)
            queues = [nc.sync, nc.scalar]
            s_ = idxw[0:16]
            src = bass.AP(tensor=s_.tensor, offset=s_.offset,
                          ap=[s_.ap[0], [1, KK * 64]])
            for g in range(1, 8):
                d_ = idxw[16 * g:16 * g + 16]
                dst = bass.AP(tensor=d_.tensor, offset=d_.offset,
                              ap=[d_.ap[0], [1, KK * 64]])
                queues[g % 2].dma_start(out=dst, in_=src)
            return idxw

        wcs, idxws = [], []
        oms, saves = [], []
        pe_warmup()
        for ph in range(2):
            om = om_conv(ph)
            idxf, saved = idx_math_crit(ph, om)
            idxws.append(idx_wrap(ph, idxf))
            oms.append(om)
            saves.append(saved)
        for ph in range(2):
            wcs.append(idx_math_wc(ph, oms[ph], saves[ph]))

        # ---------- taps: gather, diag blend, main GEMM ----------
        for ph in range(2):
            wc = wcs[ph]
            idxw = idxws[ph]
            psum_out = [outp_pool.tile([128, PH_PIX], f32, name=f"po{ph}{o2}",
                                       tag=f"po{o2}")
                        for o2 in range(2)]
            for k in range(KK):
                G = g_pool.tile([128, 8, 1024], GDT, name=f"G{ph}_{k}", tag="G")
                nc.gpsimd.dma_gather(
                    G[:], xblk_ap, idxw[:, k], PH_PIX, PH_PIX,
                    elem_size=1024, elem_step=1024, queue_num=0)
                diags = []
                for q in range(8):
                    d4 = []
                    for c4 in range(4):
                        diag = diag_pool.tile([128, 128], f16, name="diag", tag="diag")
                        wsl = wc[c4][:, q, k:k + 1]
                        nc.vector.tensor_scalar(diag[:], ident_t[:], wsl, None, Alu.mult)
                        d4.append(diag)
                    diags.append(d4)
                # all 4 transpose-blend groups first, then all main matmuls:
                # the colsT copies overlap later groups' blends instead of
                # stalling the PE at each group boundary
                colsTs = []
                for chh in range(2):
                    for half in range(2):
                        pc = colp_pool.tile([128, 512], f32, name="pc",
                                            tag="pc")
                        for qq in range(4):
                            q = half * 4 + qq
                            for c4 in range(4):
                                nc.tensor.matmul(
                                    pc[:, qq * 128:(qq + 1) * 128],
                                    lhsT=G[:, q, c4 * 256 + chh * 128:
                                           c4 * 256 + chh * 128 + 128],
                                    rhs=diags[q][c4][:],
                                    start=(c4 == 0), stop=(c4 == 3))
                        colsT = cols_pool.tile([128, 512], f16, name="colsT",
                                               tag="colsT")
                        nc.scalar.activation(colsT[:], pc[:], Act.Copy)
                        colsTs.append((chh, half, colsT))
                for chh, half, colsT in colsTs:
                    t = k * 2 + chh
                    for o2 in range(2):
                        nc.tensor.matmul(
                            psum_out[o2][:, half * 512:(half + 1) * 512],
                            lhsT=wT[:, t * 256 + o2 * 128:
                                    t * 256 + o2 * 128 + 128],
                            rhs=colsT[:],
                            start=(t == 0), stop=(t == 17))

            # bias + psum->sbuf copies run on Act (o2=0) and DVE (o2=1) in
            # parallel; halves pipeline into two DMA queues
            for o2 in range(2):
                osb = outs_pool.tile([128, PH_PIX], f16, name=f"osb{ph}{o2}",
                                     tag=f"osb{o2}")
                if o2 == 0:
                    nc.scalar.activation(osb[:], psum_out[o2][:],
                                         Act.Identity, bias=bias_t[:, 0:1])
                else:
                    nc.vector.tensor_scalar(osb[:], psum_out[o2][:],
                                            bias_t[:, 1:2], None, Alu.add)
                od = out_d.ap()
                dst = bass.AP(tensor=od.tensor,
                              offset=od.offset + o2 * 128 * 2048 + ph * PH_PIX,
                              ap=[[2048, 128], [1, PH_PIX]])
                [nc.sync, nc.scalar][o2].dma_start(out=dst, in_=osb[:])

    if not skip_compile:
        nc.compile()
    return nc


_NC_CACHE = {}


def _get_nc():
    if "nc" not in _NC_CACHE:
        _NC_CACHE["nc"] = build_nc()
    return _NC_CACHE["nc"]


def kernel(x, weight, bias, w_om, b_om):
    x = np.ascontiguousarray(np.asarray(x, dtype=np.float32))
    weight = np.asarray(weight, dtype=np.float32)
    bias = np.asarray(bias, dtype=np.float32)
    w_om = np.asarray(w_om, dtype=np.float32)
    b_om = np.asarray(b_om, dtype=np.float32)

    per_core = host_prep(x, weight, bias, w_om, b_om)
    in_maps = []
    for pc in per_core:
        in_maps.append({
            "xblk": pc["xblk"],
            "xp0a": pc["xp0a"], "xp1a": pc["xp1a"],
            "xp0b": pc["xp0b"], "xp1b": pc["xp1b"],
            "wT": pc["wT"], "womT": pc["womT"],
            "bias": pc["bias"], "bom": pc["bom"],
            "gyk": pc["gyk"], "gxk": pc["gxk"],
            "ident": pc["ident"], "identf32": pc["identf32"],
            "cconst": pc["cconst"],
        })

    nc = _get_nc()
    res = run_bass_kernel_spmd(nc, in_maps, list(range(NCORES)))

    out = np.zeros((B, O, H, W), dtype=np.float32)
    for core in range(NCORES):
        b, h = divmod(core, 2)
        oc = res.results[core]["out"].astype(np.float32)
        out[b, :, 32 * h:32 * h + 32, :] = oc.reshape(O, ROWS_PER_CORE, W)
    return out


# revision 3
# speedup vs baseline: 1.0047x; 1.0047x over previous
"""DCNv2 (deformable conv v2) Trainium2 kernel — 8-core SPMD, batch x
H-half sharding.

v3 over v2 baseline: float8_e3m4 corner-block gather (halves gather DMA,
36MB -> 18MB/core) with mixed-dtype fp8xf16 diag-blend matmuls; idx wrap via
PE transposes + DVE permute instead of 11 serial 2B-descriptor DMAs; PE
warmup so the om conv runs at full pstate; per-tap blend groups emitted
before main GEMMs (colsT copies overlap later blends); om conv input loads
phase-split and ordered by critical path; outputs copied Act/DVE in
parallel. rel_err ~1.25e-2 (fp8 x-quantization, deterministic).
"""
import numpy as np
from contextlib import ExitStack

import concourse.bass as bass
import concourse.mybir as mybir
import concourse.tile as tile
from concourse import bacc
from concourse.bass_utils import run_bass_kernel_spmd

F16 = np.float16
GATHER_FP8 = True   # float8_e3m4 xblk + mixed-dtype blend matmuls
try:
    import ml_dtypes
    E3M4 = ml_dtypes.float8_e3m4
except ImportError:
    E3M4 = None

B, C, H, W = 4, 256, 64, 64
O = 256
K = 3
KK = 9
NCORES = 8
ROWS_PER_CORE = 32
PIX_PER_CORE = ROWS_PER_CORE * W
PH_PIX = 1024          # pixels per phase (16 rows)
NBLK_Y = 37            # block-grid rows per core
NBLK_X = 66
NBLK = NBLK_Y * NBLK_X  # 2442

f16 = mybir.dt.float16
f8e3 = mybir.dt.float8e3
f32 = mybir.dt.float32
_FP8 = GATHER_FP8 and E3M4 is not None
GDT = f8e3 if _FP8 else f16
GNP = E3M4 if _FP8 else F16
i16 = mybir.dt.int16
i32 = mybir.dt.int32
Alu = mybir.AluOpType
Act = mybir.ActivationFunctionType


def host_prep(x, weight, bias, w_om, b_om):
    """Per-core inputs. x:[B,C,H,W] f32, weight:[O,C,3,3], bias:[O],
    w_om:[27,C,3,3], b_om:[27]."""
    # weight reorder: wT_sb[p, t*256 + o] = weight[o, ch, ky, kx],
    # t = k*2 + chh, ch = chh*128 + p
    wr = weight.reshape(O, C, KK).transpose(2, 1, 0).reshape(KK * C, O)
    wT_sb = wr.reshape(18, 128, O).transpose(1, 0, 2).reshape(128, 18 * O).astype(F16)
    # om weights, chh-major: t2 = chh*9 + k (chh-major accumulation order)
    womr = w_om.reshape(27, C, KK).transpose(2, 1, 0).reshape(KK, 2, 128, 27)
    womT_sb = womr.transpose(2, 1, 0, 3).reshape(128, 18 * 27).astype(F16)
    bias_sb = bias.reshape(2, 128).T.astype(np.float32)         # [128, 2]
    bom_sb = b_om.reshape(1, 27).astype(F16)                    # [1, 27]
    ident16 = np.eye(128, dtype=F16)

    per_core = []
    for core in range(NCORES):
        b, h = divmod(core, 2)
        xb = x[b]  # [C, H, W]
        # xp: 66x66 zero-padded channel-major f16, rows [32h-1, 32h+33) of the
        # padded frame (covers the core's 32 output rows + om-conv halo)
        xp = np.zeros((C, 66, 66), dtype=F16)
        xp[:, 1:65, 1:65] = xb.astype(F16)
        xpc = xp[:, 32 * h:32 * h + 34, :]
        xpad16 = np.ascontiguousarray(xpc).reshape(2, 128, 34 * 66)
        # xblk: 2x2 corner blocks. xblk[Yb, X, a, b2, c] = xpp[c, Yb+R0+1+a, X+b2]
        # where block (Yb, X) is anchor (y0, x0) = (R0+Yb, X-1).
        R0 = max(-1, 32 * h - 4)
        xpp = np.zeros((C, 67, 67), dtype=F16)
        xpp[:, 1:65, 1:65] = xb.astype(F16)
        xblk = np.empty((NBLK_Y, NBLK_X, 2, 2, C), dtype=F16)
        for a in range(2):
            for b2 in range(2):
                xblk[:, :, a, b2, :] = xpp[:, R0 + 1 + a:R0 + 1 + a + NBLK_Y,
                                           b2:b2 + NBLK_X].transpose(1, 2, 0)
        xblk = np.ascontiguousarray(xblk).reshape(NBLK, 1024).astype(GNP)
        cconst = np.full((128, 1), -(66.0 * (64 + R0) + 63.0), dtype=np.float32)
        # grids (absolute y incl 32h): raster pixel j = 64 r + c within phase;
        # part = 64(r%2)+c, slot = r//2
        part = np.arange(128)
        kk = np.arange(KK)
        ky, kx = kk // K, kk % K
        r2 = part // 64
        ccol = part % 64
        slot = np.arange(8)
        gyk = np.zeros((128, 2, 8, KK), dtype=np.float32)
        for ph in range(2):
            gyk[:, ph, :, :] = ((32 * h + 16 * ph + r2 - 1)[:, None, None]
                                + 2 * slot[None, :, None] + ky[None, None, :])
        gxk = np.broadcast_to((ccol - 1)[:, None, None] + kx[None, None, :],
                              (128, 8, KK)).astype(np.float32).copy()
        xp0 = xpad16[0].reshape(128, 34, 66)
        xp1 = xpad16[1].reshape(128, 34, 66)
        per_core.append(dict(
            xblk=xblk,
            xp0a=np.ascontiguousarray(xp0[:, :20]),
            xp1a=np.ascontiguousarray(xp1[:, :20]),
            xp0b=np.ascontiguousarray(xp0[:, 20:]),
            xp1b=np.ascontiguousarray(xp1[:, 20:]),
            wT=wT_sb, womT=womT_sb, bias=bias_sb, bom=bom_sb,
            gyk=gyk, gxk=gxk, ident=ident16, identf32=np.eye(128, dtype=np.float32),
            cconst=cconst,
        ))
    return per_core


def build_nc(skip_compile=False):
    nc = bacc.Bacc("TRN2", target_bir_lowering=False, debug=False, num_devices=8)

    xblk_d = nc.dram_tensor("xblk", [NBLK, 1024], GDT, kind="ExternalInput")
    xpa_d = [nc.dram_tensor(f"xp{i}a", [128, 20, 66], f16, kind="ExternalInput")
             for i in range(2)]
    xpb_d = [nc.dram_tensor(f"xp{i}b", [128, 14, 66], f16, kind="ExternalInput")
             for i in range(2)]
    wT_d = nc.dram_tensor("wT", [128, 18 * 256], f16, kind="ExternalInput")
    womT_d = nc.dram_tensor("womT", [128, 18 * 27], f16, kind="ExternalInput")
    bias_d = nc.dram_tensor("bias", [128, 2], f32, kind="ExternalInput")
    bom_d = nc.dram_tensor("bom", [1, 27], f16, kind="ExternalInput")
    gyk_d = nc.dram_tensor("gyk", [128, 2, 8, KK], f32, kind="ExternalInput")
    gxk_d = nc.dram_tensor("gxk", [128, 8, KK], f32, kind="ExternalInput")
    ident_d = nc.dram_tensor("ident", [128, 128], f16, kind="ExternalInput")
    identf32_d = nc.dram_tensor("identf32", [128, 128], f32, kind="ExternalInput")
    cconst_d = nc.dram_tensor("cconst", [128, 1], f32, kind="ExternalInput")
    out_d = nc.dram_tensor("out", [256, 2048], f16, kind="ExternalOutput")

    # block view for the gather: row i -> 1024 contiguous f16 (2x2x256 block)
    xblk_ap = bass.AP(tensor=xblk_d, offset=0, ap=[[1024, NBLK], [1, 1024]])

    with tile.TileContext(nc) as tc, ExitStack() as ctx:
        const = ctx.enter_context(tc.tile_pool(name="const", bufs=1))
        xp = [const.tile([128, 34, 66], f16, name=f"xp{i}", tag=f"xp{i}") for i in range(2)]
        womT = const.tile([128, 18 * 27], f16, name="womT", tag="womT")
        gyk_t = const.tile([128, 2, 8, KK], f32, name="gyk", tag="gyk")
        gxk_t = const.tile([128, 8, KK], f32, name="gxk", tag="gxk")
        ident_t = const.tile([128, 128], f16, name="ident", tag="ident")
        identf32 = const.tile([128, 128], f32, name="identf32", tag="identf32")
        bias_t = const.tile([128, 2], f32, name="bias", tag="bias")
        cconst_t = const.tile([128, 1], f32, name="cconst", tag="cconst")
        bom_t = const.tile([1, 27], f16, name="bom", tag="bom")
        wT = const.tile([128, 18 * 256], f16, name="wT", tag="wT")
        ones_t = const.tile([1, 128], f16, name="ones", tag="ones")
        ones_px = const.tile([1, PH_PIX], f16, name="onespx", tag="onespx")
        # critical-path loads first: om ph0 needs xp0a+womT (chh-major), then
        # xp1a; bom gates the per-pt bias matmul; gyk/gxk/cconst gate idx math
        nc.sync.dma_start(out=xp[0][:, :20], in_=xpa_d[0].ap())
        nc.scalar.dma_start(out=womT[:], in_=womT_d.ap())
        nc.scalar.dma_start(out=bom_t[:], in_=bom_d.ap())
        nc.sync.dma_start(out=xp[1][:, :20], in_=xpa_d[1].ap())
        nc.scalar.dma_start(out=gyk_t[:], in_=gyk_d.ap())
        nc.scalar.dma_start(out=gxk_t[:], in_=gxk_d.ap())
        nc.scalar.dma_start(out=cconst_t[:], in_=cconst_d.ap())
        nc.scalar.dma_start(out=ident_t[:], in_=ident_d.ap())
        nc.scalar.dma_start(out=identf32[:], in_=identf32_d.ap())
        nc.sync.dma_start(out=xp[0][:, 20:], in_=xpb_d[0].ap())
        nc.sync.dma_start(out=xp[1][:, 20:], in_=xpb_d[1].ap())
        nc.scalar.dma_start(out=bias_t[:], in_=bias_d.ap())
        nc.scalar.dma_start(out=wT[:], in_=wT_d.ap())
        nc.gpsimd.memset(ones_t[:], 1.0)
        nc.gpsimd.memset(ones_px[:], 1.0)
        warm = const.tile([1, 128], f16, name="warm", tag="warm")
        nc.gpsimd.memset(warm[:], 0.0)

        # psum pools: colp 4 (om psum borrows the first two ring slots, freed
        # after the om->sbuf copies) + outp 4 = 8 banks
        colp_pool = ctx.enter_context(tc.tile_pool(name="colp", bufs=4, space="PSUM"))
        outp_pool = ctx.enter_context(tc.tile_pool(name="outp", bufs=1, space="PSUM"))

        math_pool = ctx.enter_context(tc.tile_pool(name="math", bufs=2))
        idxw_pool = ctx.enter_context(tc.tile_pool(name="idxw", bufs=2))
        g_pool = ctx.enter_context(tc.tile_pool(name="g", bufs=4))
        diag_pool = ctx.enter_context(tc.tile_pool(name="diag", bufs=48))
        cols_pool = ctx.enter_context(tc.tile_pool(name="cols", bufs=8))
        outs_pool = ctx.enter_context(tc.tile_pool(name="outs", bufs=2))

        # ---------- per-phase: om conv -> offset math -> idx -> gathers ----
        def pe_warmup():
            """~3us of dummy matmuls so om starts at full PE pstate."""
            wp = colp_pool.tile([128, 512], f32, name="warmp", tag="pc")
            for i in range(15):
                nc.tensor.matmul(wp[:, 0:128], lhsT=warm[:], rhs=warm[:],
                                 start=True, stop=True, skip_group_check=True)

        def om_conv(ph):
            """Row-paired om conv, chh-major accumulation (womT t2 = chh*9+k).
            psum_om[64*r2+c, pt, :] accumulates over (chh, k)."""
            pcb = colp_pool.tile([128, 512], f32, name=f"pom{ph}", tag="pc")
            b_ = pcb[:]
            psum_om = bass.AP(tensor=b_.tensor, offset=b_.offset,
                              ap=[b_.ap[0], [27, 8], [1, 27]])
            # contiguous accumulation per psum region (interleaved groups
            # corrupt psum on HW); chh-major t2 order so chh=0 runs first
            for pt in range(8):
                for r2 in range(2):
                    for t2 in range(18):
                        chh, k = divmod(t2, 9)
                        ky, kx = k // 3, k % 3
                        r0 = 16 * ph + 2 * pt + r2 + ky
                        lhsT = xp[chh][:, r0, kx:kx + 64]
                        nc.tensor.matmul(
                            psum_om[64 * r2:64 * r2 + 64, pt],
                            lhsT=lhsT,
                            rhs=womT[:, t2 * 27:(t2 + 1) * 27],
                            start=(t2 == 0), stop=False)
                nc.tensor.matmul(psum_om[:, pt], lhsT=ones_t[:],
                                 rhs=bom_t[:], start=False, stop=True)
            om = math_pool.tile([128, 8, 27], f32, name=f"ompm{ph}", tag=f"ompm{ph}")
            nc.scalar.activation(om[:], psum_om, Act.Copy)
            return om

        def idx_math_crit(ph, ompm):
            """Critical path: offsets -> floor -> block idx (i16, k-major).
            Only what the gather wraps need; lerp weights computed later."""
            def mt(tag):
                return math_pool.tile([128, 8, KK], f32, name=tag + str(ph),
                                      tag=tag + str(ph))

            dy = ompm[:, :, 0:KK]
            dx = ompm[:, :, KK:2 * KK]
            ty, tx_ = mt("ty"), mt("tx")
            y064, x064 = mt("y064"), mt("x064")
            ya, xa, idxf = mt("ya"), mt("xa"), mt("idxf")

            V = nc.vector
            V.tensor_tensor(ty[:], dy, gyk_t[:, ph], Alu.add)
            V.tensor_scalar_add(ty[:], ty[:], 64.0)
            V.tensor_tensor(tx_[:], dx, gxk_t[:], Alu.add)
            V.tensor_scalar_add(tx_[:], tx_[:], 64.0)
            yi32 = math_pool.tile([128, 8, KK], i32, name=f"yi32{ph}", tag=f"yi32{ph}")
            xi32 = math_pool.tile([128, 8, KK], i32, name=f"xi32{ph}", tag=f"xi32{ph}")
            yif, xif = mt("yif"), mt("xif")
            gq, gqx = mt("gq"), mt("gqx")
            V.tensor_copy(yi32[:], ty[:])
            V.tensor_copy(yif[:], yi32[:])
            V.tensor_tensor(gq[:], yif[:], ty[:], Alu.is_gt)
            V.tensor_tensor(y064[:], yif[:], gq[:], Alu.subtract)
            V.tensor_copy(xi32[:], tx_[:])
            V.tensor_copy(xif[:], xi32[:])
            V.tensor_tensor(gqx[:], xif[:], tx_[:], Alu.is_gt)
            V.tensor_tensor(x064[:], xif[:], gqx[:], Alu.subtract)
            # block idx = (ya-64-R0)*66 + (xa-63) = 66*ya + xa + cconst
            V.tensor_scalar(ya[:], y064[:], 63.0, 128.0, Alu.max, Alu.min)
            V.tensor_scalar(xa[:], x064[:], 63.0, 128.0, Alu.max, Alu.min)
            V.scalar_tensor_tensor(idxf[:], ya[:], 66.0, xa[:], Alu.mult, Alu.add)
            V.tensor_scalar(idxf[:], idxf[:], cconst_t[:, 0:1], None, Alu.add)
            return idxf, (ty, tx_, y064, x064)

        def idx_math_wc(ph, ompm, saved):
            """Deferred: bilinear corner weights x mask (needed at first
            gather completion, ~6us after the wraps)."""
            ty, tx_, y064, x064 = saved

            def mt(tag):
                return math_pool.tile([128, 8, KK], f32, name=tag + str(ph),
                                      tag=tag + str(ph))

            ml = ompm[:, :, 2 * KK:3 * KK]
            fry, frx, m_t = mt("fry"), mt("frx"), mt("m")
            s0y, g0, ay0, ay0m = mt("s0y"), mt("g0"), mt("ay0"), mt("ay0m")
            g1, ay1, ay1m = mt("g1"), mt("ay1"), mt("ay1m")
            s0x, g0x, bx0, g1x, bx1 = mt("s0x"), mt("g0x"), mt("bx0"), mt("g1x"), mt("bx1")
            wc = [mt(f"wc{i}") for i in range(4)]

            V = nc.vector
            V.tensor_tensor(fry[:], ty[:], y064[:], Alu.subtract)
            V.tensor_tensor(frx[:], tx_[:], x064[:], Alu.subtract)
            nc.scalar.activation(m_t[:], ml, Act.Sigmoid)
            V.tensor_scalar(s0y[:], fry[:], -1.0, 1.0, Alu.mult, Alu.add)
            V.scalar_tensor_tensor(g0[:], y064[:], 64.0, s0y[:], Alu.is_ge, Alu.mult)
            V.scalar_tensor_tensor(ay0[:], y064[:], 127.0, g0[:], Alu.is_le, Alu.mult)
            V.tensor_tensor(ay0m[:], ay0[:], m_t[:], Alu.mult)
            V.scalar_tensor_tensor(g1[:], y064[:], 63.0, fry[:], Alu.is_ge, Alu.mult)
            V.scalar_tensor_tensor(ay1[:], y064[:], 126.0, g1[:], Alu.is_le, Alu.mult)
            V.tensor_tensor(ay1m[:], ay1[:], m_t[:], Alu.mult)
            V.tensor_scalar(s0x[:], frx[:], -1.0, 1.0, Alu.mult, Alu.add)
            V.scalar_tensor_tensor(g0x[:], x064[:], 64.0, s0x[:], Alu.is_ge, Alu.mult)
            V.scalar_tensor_tensor(bx0[:], x064[:], 127.0, g0x[:], Alu.is_le, Alu.mult)
            V.scalar_tensor_tensor(g1x[:], x064[:], 63.0, frx[:], Alu.is_ge, Alu.mult)
            V.scalar_tensor_tensor(bx1[:], x064[:], 126.0, g1x[:], Alu.is_le, Alu.mult)
            V.tensor_tensor(wc[0][:], ay0m[:], bx0[:], Alu.mult)
            V.tensor_tensor(wc[1][:], ay0m[:], bx1[:], Alu.mult)
            V.tensor_tensor(wc[2][:], ay1m[:], bx0[:], Alu.mult)
            V.tensor_tensor(wc[3][:], ay1m[:], bx1[:], Alu.mult)
            return wc

        def idx_wrap(ph, idxf):
            """idxw[pw, k, s, pq] = round(idxf[16 pq + pw, s, k]) via PE
            transposes (PE is idle here) + 2 DVE permute-copies, then 7
            replica DMAs for the other gather-core windows."""
            idxw = idxw_pool.tile([128, KK, 8, 8], i16, name=f"idxw{ph}",
                                  tag=f"idxw{ph}")
            # T1 = idxf^T : [72 (s,k), 128 pixels]
            t1p = colp_pool.tile([72, 128], f32, name=f"t1p{ph}", tag="pc")
            i_ = idxf[:]
            src128 = bass.AP(tensor=i_.tensor, offset=i_.offset,
                             ap=[i_.ap[0], [1, 72]])
            nc.tensor.transpose(t1p[:], src128, identf32[:])
            t1s = math_pool.tile([72, 128], f32, name=f"t1s{ph}",
                                 tag=f"t1s{ph}")
            nc.scalar.activation(t1s[:], t1p[:], Act.Copy)
            # T2_pq = T1[:, 16 pq : 16 pq+16]^T : [16 pw, 72 (s,k)]
            t2 = [colp_pool.tile([16, 4, 72], f32, name=f"t2{ph}{i}", tag="pc")
                  for i in range(2)]
            for pq in range(8):
                nc.tensor.transpose(t2[pq // 4][:, pq % 4],
                                    t1s[:, 16 * pq:16 * pq + 16],
                                    identf32[0:72, 0:72])
            # assemble window 0: dst (k, s, pq) <- src (pq, s, k), cast i16
            for i in range(2):
                s_ = t2[i][:]
                srcap = bass.AP(tensor=s_.tensor, offset=s_.offset,
                                ap=[s_.ap[0], [72, 4], [9, 8], [1, 9]])
                d_ = idxw[0:16]
                dstap = bass.AP(tensor=d_.tensor, offset=d_.offset + 4 * i,
                                ap=[d_.ap[0], [1, 4], [8, 8], [64, KK]])
                nc.vector.tensor_copy(dstap, srcap)
            # each of the 8 gather cores reads its own 16-partition idx
            # window: replicate group 0 -> 1..7 (7 parallel DMAs)
            # 5 replicas on the two HWDGE queues + 2 on the (currently
            # idle) Pool SWDGE queue: descriptor gen waits for all replicas
            # anyway, so the extra queue shortens the last-replica time
            queues = [nc.sync, nc.scalar, nc.sync, nc.scalar, nc.gpsimd,
                      nc.sync, nc.gpsimd]
            s_ = idxw[0:16]
            src = bass.AP(tensor=s_.tensor, offset=s_.offset,
                          ap=[s_.ap[0], [1, KK * 64]])
            for g in range(1, 8):
                d_ = idxw[16 * g:16 * g + 16]
                dst = bass.AP(tensor=d_.tensor, offset=d_.offset,
                              ap=[d_.ap[0], [1, KK * 64]])
                queues[g - 1].dma_start(out=dst, in_=src)
            return idxw

        wcs, idxws = [], []
        oms, saves = [], []
        pe_warmup()
        for ph in range(2):
            om = om_conv(ph)
            idxf, saved = idx_math_crit(ph, om)
            idxws.append(idx_wrap(ph, idxf))
            oms.append(om)
            saves.append(saved)
        for ph in range(2):
            wcs.append(idx_math_wc(ph, oms[ph], saves[ph]))

        # ---------- taps: gather, diag blend, main GEMM ----------
        for ph in range(2):
            wc = wcs[ph]
            idxw = idxws[ph]
            psum_out = [outp_pool.tile([128, PH_PIX], f32, name=f"po{ph}{o2}",
                                       tag=f"po{o2}")
                        for o2 in range(2)]
            for k in range(KK):
                G = g_pool.tile([128, 8, 1024], GDT, name=f"G{ph}_{k}", tag="G")
                nc.gpsimd.dma_gather(
                    G[:], xblk_ap, idxw[:, k], PH_PIX, PH_PIX,
                    elem_size=1024, elem_step=1024, queue_num=0)
                diags = []
                for q in range(8):
                    d4 = []
                    for c4 in range(4):
                        diag = diag_pool.tile([128, 128], f16, name="diag", tag="diag")
                        wsl = wc[c4][:, q, k:k + 1]
                        nc.vector.tensor_scalar(diag[:], ident_t[:], wsl, None, Alu.mult)
                        d4.append(diag)
                    diags.append(d4)
                # all 4 transpose-blend groups first, then all main matmuls:
                # the colsT copies overlap later groups' blends instead of
                # stalling the PE at each group boundary
                colsTs = []
                for chh in range(2):
                    for half in range(2):
                        pc = colp_pool.tile([128, 512], f32, name="pc",
                                            tag="pc")
                        for qq in range(4):
                            q = half * 4 + qq
                            for c4 in range(4):
                                nc.tensor.matmul(
                                    pc[:, qq * 128:(qq + 1) * 128],
                                    lhsT=G[:, q, c4 * 256 + chh * 128:
                                           c4 * 256 + chh * 128 + 128],
                                    rhs=diags[q][c4][:],
                                    start=(c4 == 0), stop=(c4 == 3))
                        colsT = cols_pool.tile([128, 512], f16, name="colsT",
                                               tag="colsT")
                        nc.scalar.activation(colsT[:], pc[:], Act.Copy)
                        colsTs.append((chh, half, colsT))
                for chh, half, colsT in colsTs:
                    t = k * 2 + chh
                    for o2 in range(2):
                        nc.tensor.matmul(
                            psum_out[o2][:, half * 512:(half + 1) * 512],
                            lhsT=wT[:, t * 256 + o2 * 128:
                                    t * 256 + o2 * 128 + 128],
                            rhs=colsT[:],
                            start=(t == 0), stop=(t == 17))

            # bias + psum->sbuf copies run on Act (o2=0) and DVE (o2=1) in
            # parallel; halves pipeline into two DMA queues
            for o2 in range(2):
                osb = outs_pool.tile([128, PH_PIX], f16, name=f"osb{ph}{o2}",
                                     tag=f"osb{o2}")
                if o2 == 0:
                    nc.scalar.activation(osb[:], psum_out[o2][:],
                                         Act.Identity, bias=bias_t[:, 0:1])
                else:
                    nc.vector.tensor_scalar(osb[:], psum_out[o2][:],
                                            bias_t[:, 1:2], None, Alu.add)
                od = out_d.ap()
                dst = bass.AP(tensor=od.tensor,
                              offset=od.offset + o2 * 128 * 2048 + ph * PH_PIX,
                              ap=[[2048, 128], [1, PH_PIX]])
                [nc.sync, nc.scalar][o2].dma_start(out=dst, in_=osb[:])

    if not skip_compile:
        nc.compile()
    return nc


_NC_CACHE = {}


def _get_nc():
    if "nc" not in _NC_CACHE:
        _NC_CACHE["nc"] = build_nc()
    return _NC_CACHE["nc"]


def kernel(x, weight, bias, w_om, b_om):
    x = np.ascontiguousarray(np.asarray(x, dtype=np.float32))
    weight = np.asarray(weight, dtype=np.float32)
    bias = np.asarray(bias, dtype=np.float32)
    w_om = np.asarray(w_om, dtype=np.float32)
    b_om = np.asarray(b_om, dtype=np.float32)

    per_core = host_prep(x, weight, bias, w_om, b_om)
    in_maps = []
    for pc in per_core:
        in_maps.append({
            "xblk": pc["xblk"],
            "xp0a": pc["xp0a"], "xp1a": pc["xp1a"],
            "xp0b": pc["xp0b"], "xp1b": pc["xp1b"],
            "wT": pc["wT"], "womT": pc["womT"],
            "bias": pc["bias"], "bom": pc["bom"],
            "gyk": pc["gyk"], "gxk": pc["gxk"],
            "ident": pc["ident"], "identf32": pc["identf32"],
            "cconst": pc["cconst"],
        })

    nc = _get_nc()
    res = run_bass_kernel_spmd(nc, in_maps, list(range(NCORES)))

    out = np.zeros((B, O, H, W), dtype=np.float32)
    for core in range(NCORES):
        b, h = divmod(core, 2)
        oc = res.results[core]["out"].astype(np.float32)
        out[b, :, 32 * h:32 * h + 32, :] = oc.reshape(O, ROWS_PER_CORE, W)
    return out
